# revision 2
# baseline (speedup 1.0000x reference)
"""Trainium2 Bass kernel for the controlled-unitary problem.

reference semantics (control=0, num_qubits=13, dim=8192):
    mask bit = 1 << 12, so columns/rows with that bit set are idx 4096..8191.
    out[:, c0] = state[:, c0]                       (control bit off: untouched)
    out[:, c1] = state[:, c1] @ target[c1, c1]      (controlled unitary)

Device work: complex [256,4096] @ [4096,4096] GEMM = 4 real GEMMs.
Sharding: output columns of the GEMM split 8 ways (each core gets a
[4096, 512] slab of the target block, every weight byte moves once).
Per core, 8 PSUM banks hold the 4 real products x 2 M-tiles, accumulated
over 32 K-tiles of 128; VectorE combines real/imag; results DMA out.
"""

import os

import numpy as np

BATCH = 256
DIM = 8192
HALF = 4096
N_CORES = 8
NSH = HALF // N_CORES  # 512 output columns per core
KT = HALF // 128  # 32 k-tiles
CH = 4  # k-tiles per DMA chunk
MT = BATCH // 128  # 2 m-tiles

# matmul dtype: "float32r" = full-rate fp32 path, "float16" = half traffic
DT_NAME = os.environ.get("KERNEL_DT", "float32r")

_CACHE = {}


def _np_dtype(dt_name):
    return np.float16 if dt_name == "float16" else np.float32


def _build(dt_name):
    import concourse.mybir as mybir
    import concourse.tile as tile
    from concourse import bacc

    DT = getattr(mybir.dt, dt_name)
    F32 = mybir.dt.float32

    nc = bacc.Bacc("TRN2", target_bir_lowering=False, debug=False,
                   num_devices=N_CORES)

    a_r = nc.dram_tensor("a_r", [128, KT, BATCH], DT, kind="ExternalInput")
    a_i = nc.dram_tensor("a_i", [128, KT, BATCH], DT, kind="ExternalInput")
    b_r = nc.dram_tensor("b_r", [128, KT, NSH], DT, kind="ExternalInput")
    b_i = nc.dram_tensor("b_i", [128, KT, NSH], DT, kind="ExternalInput")
    c_r = nc.dram_tensor("c_r", [BATCH, NSH], F32, kind="ExternalOutput")
    c_i = nc.dram_tensor("c_i", [BATCH, NSH], F32, kind="ExternalOutput")

    with tile.TileContext(nc) as tc:
        with (
            tc.tile_pool(name="ap", bufs=3) as ap_pool,
            tc.tile_pool(name="bp", bufs=3) as bp_pool,
            tc.tile_pool(name="op", bufs=2) as o_pool,
            tc.tile_pool(name="ps", bufs=1, space="PSUM") as ps_pool,
        ):
            ps = {}
            for m in range(MT):
                for comp in ("rr", "ri", "ir", "ii"):
                    ps[(m, comp)] = ps_pool.tile(
                        [128, NSH], F32, name=f"ps_{m}_{comp}"
                    )

            for c in range(KT // CH):
                ar_t = ap_pool.tile([128, CH, BATCH], DT, name="ar")
                ai_t = ap_pool.tile([128, CH, BATCH], DT, name="ai")
                br_t = bp_pool.tile([128, CH, NSH], DT, name="br")
                bi_t = bp_pool.tile([128, CH, NSH], DT, name="bi")
                ksl = slice(c * CH, (c + 1) * CH)
                nc.sync.dma_start(ar_t[:], a_r[:, ksl, :])
                nc.sync.dma_start(ai_t[:], a_i[:, ksl, :])
                nc.sync.dma_start(br_t[:], b_r[:, ksl, :])
                nc.sync.dma_start(bi_t[:], b_i[:, ksl, :])
                for kk in range(CH):
                    k = c * CH + kk
                    start = k == 0
                    stop = k == KT - 1
                    for m in range(MT):
                        msl = slice(m * 128, (m + 1) * 128)
                        lr = ar_t[:, kk, msl]
                        li = ai_t[:, kk, msl]
                        rr = br_t[:, kk, :]
                        ri = bi_t[:, kk, :]
                        nc.tensor.matmul(ps[(m, "rr")][:], lr, rr,
                                         start=start, stop=stop)
                        nc.tensor.matmul(ps[(m, "ri")][:], lr, ri,
                                         start=start, stop=stop)
                        nc.tensor.matmul(ps[(m, "ir")][:], li, rr,
                                         start=start, stop=stop)
                        nc.tensor.matmul(ps[(m, "ii")][:], li, ri,
                                         start=start, stop=stop)

            for m in range(MT):
                msl = slice(m * 128, (m + 1) * 128)
                t_ii = o_pool.tile([128, NSH], F32, name="t_ii")
                t_ir = o_pool.tile([128, NSH], F32, name="t_ir")
                out_r = o_pool.tile([128, NSH], F32, name="out_r")
                out_i = o_pool.tile([128, NSH], F32, name="out_i")
                nc.any.tensor_copy(t_ii[:], ps[(m, "ii")][:])
                nc.any.tensor_copy(t_ir[:], ps[(m, "ir")][:])
                nc.vector.tensor_tensor(out_r[:], ps[(m, "rr")][:], t_ii[:],
                                        mybir.AluOpType.subtract)
                nc.vector.tensor_tensor(out_i[:], ps[(m, "ri")][:], t_ir[:],
                                        mybir.AluOpType.add)
                nc.sync.dma_start(c_r[msl, :], out_r[:])
                nc.sync.dma_start(c_i[msl, :], out_i[:])

    nc.compile()
    return nc


def _get_nc(dt_name):
    if dt_name not in _CACHE:
        _CACHE[dt_name] = _build(dt_name)
    return _CACHE[dt_name]


def _pack_kxm(mat_t, np_dt):
    # mat_t: [4096, F] (k-major) -> [128, KT, F] with k = kt*128 + p
    f = mat_t.shape[1]
    return np.ascontiguousarray(
        mat_t.reshape(KT, 128, f).transpose(1, 0, 2).astype(np_dt)
    )


def run_device(A, B, dt_name=DT_NAME, trace=False):
    """A: [256, 4096] complex64, B: [4096, 4096] complex64.
    Returns C = A @ B as [256, 4096] complex64 (and exec_time_ns if traced).
    """
    from concourse import bass_utils

    nc = _get_nc(dt_name)
    np_dt = _np_dtype(dt_name)

    at = A.T  # [4096, 256]
    a_r = _pack_kxm(np.ascontiguousarray(at.real), np_dt)
    a_i = _pack_kxm(np.ascontiguousarray(at.imag), np_dt)
    br_full = B.real
    bi_full = B.imag

    in_maps = []
    for c in range(N_CORES):
        csl = slice(c * NSH, (c + 1) * NSH)
        in_maps.append({
            "a_r": a_r,
            "a_i": a_i,
            "b_r": _pack_kxm(np.ascontiguousarray(br_full[:, csl]), np_dt),
            "b_i": _pack_kxm(np.ascontiguousarray(bi_full[:, csl]), np_dt),
        })

    res = bass_utils.run_bass_kernel_spmd(
        nc, in_maps, core_ids=list(range(N_CORES)), trace=trace
    )

    out = np.empty((BATCH, HALF), dtype=np.complex64)
    for c in range(N_CORES):
        csl = slice(c * NSH, (c + 1) * NSH)
        out.real[:, csl] = res.results[c]["c_r"]
        out.imag[:, csl] = res.results[c]["c_i"]
    return out, res


def kernel(state, target_matrix, control, num_qubits):
    state = np.asarray(state)
    target_matrix = np.asarray(target_matrix)
    control = int(control)
    num_qubits = int(num_qubits)
    dim = 1 << num_qubits

    assert state.shape == (BATCH, DIM) and dim == DIM, (
        "kernel hardcoded for [256, 8192]"
    )

    mask = 1 << (num_qubits - control - 1)
    idx = np.arange(dim)
    c1 = idx[(idx & mask) != 0]  # columns with control bit set

    if control == 0:
        A = state[:, HALF:]
        B = target_matrix[HALF:, HALF:]
    else:
        A = state[:, c1]
        B = target_matrix[np.ix_(c1, c1)]
    A = np.ascontiguousarray(A, dtype=np.complex64)
    B = np.ascontiguousarray(B, dtype=np.complex64)

    C, _ = run_device(A, B)

    out = state.astype(np.complex64, copy=True)
    out[:, c1] = C
    return out


# revision 3
# speedup vs baseline: 1.0016x; 1.0016x over previous
"""Trainium2 Bass kernel for the controlled-unitary problem.

reference semantics (control=0, num_qubits=13, dim=8192):
    mask bit = 1 << 12, so columns/rows with that bit set are idx 4096..8191.
    out[:, c0] = state[:, c0]                       (control bit off: untouched)
    out[:, c1] = state[:, c1] @ target[c1, c1]      (controlled unitary)

Device work: complex [256,4096] @ [4096,4096] GEMM = 4 real GEMMs.
Sharding: output columns of the GEMM split 8 ways (each core gets a
[4096, 512] slab of the target block; every weight byte moves once).

Per-core kernel (v2):
  - A planes a_r, a_i and a_n = -a_i (negation host-side) let the real
    part accumulate directly in PSUM: bank_r += a_r.b_r + a_n.b_i,
    bank_i += a_r.b_i + a_i.b_r  ->  4 PSUM banks (2 M-tiles x re/im),
    combine is just a PSUM->SBUF copy.
  - DMA on both HWDGE rings: A planes + outputs on nc.sync (SP ring),
    B planes on nc.scalar (ACT ring).
  - K streamed in ramped chunks (small first chunk so the PE starts
    early, big later chunks for DMA efficiency).
"""

import os

import numpy as np

BATCH = 256
DIM = 8192
HALF = 4096
N_CORES = 8
NSH = HALF // N_CORES  # 512 output columns per core
KT = HALF // 128  # 32 k-tiles
MT = BATCH // 128  # 2 m-tiles
CHUNKS = [2, 2, 4, 8, 8, 8]  # k-tiles per DMA chunk (sums to KT)
CHMAX = max(CHUNKS)

# matmul dtype: "float32r" = full-rate fp32 path, "float16" = half traffic
DT_NAME = os.environ.get("KERNEL_DT", "float16")

_CACHE = {}


def _np_dtype(dt_name):
    return np.float16 if dt_name == "float16" else np.float32


def _build(dt_name):
    import concourse.mybir as mybir
    import concourse.tile as tile
    from concourse import bacc

    DT = getattr(mybir.dt, dt_name)
    F32 = mybir.dt.float32

    nc = bacc.Bacc("TRN2", target_bir_lowering=False, debug=False,
                   num_devices=N_CORES)

    a_r = nc.dram_tensor("a_r", [128, KT, BATCH], DT, kind="ExternalInput")
    a_i = nc.dram_tensor("a_i", [128, KT, BATCH], DT, kind="ExternalInput")
    a_n = nc.dram_tensor("a_n", [128, KT, BATCH], DT, kind="ExternalInput")
    b_r = nc.dram_tensor("b_r", [128, KT, NSH], DT, kind="ExternalInput")
    b_i = nc.dram_tensor("b_i", [128, KT, NSH], DT, kind="ExternalInput")
    c_r = nc.dram_tensor("c_r", [BATCH, NSH], F32, kind="ExternalOutput")
    c_i = nc.dram_tensor("c_i", [BATCH, NSH], F32, kind="ExternalOutput")

    with tile.TileContext(nc) as tc:
        with (
            tc.tile_pool(name="ap", bufs=3) as ap_pool,
            tc.tile_pool(name="bp", bufs=3) as bp_pool,
            tc.tile_pool(name="op", bufs=2) as o_pool,
            tc.tile_pool(name="ps", bufs=1, space="PSUM") as ps_pool,
        ):
            ps = {}
            for m in range(MT):
                for comp in ("re", "im"):
                    ps[(m, comp)] = ps_pool.tile(
                        [128, NSH], F32, name=f"ps_{m}_{comp}"
                    )

            k0 = 0
            for ch in CHUNKS:
                ar_t = ap_pool.tile([128, CHMAX, BATCH], DT, name="ar")
                ai_t = ap_pool.tile([128, CHMAX, BATCH], DT, name="ai")
                an_t = ap_pool.tile([128, CHMAX, BATCH], DT, name="an")
                br_t = bp_pool.tile([128, CHMAX, NSH], DT, name="br")
                bi_t = bp_pool.tile([128, CHMAX, NSH], DT, name="bi")
                ksl = slice(k0, k0 + ch)
                # A planes + outputs ride the SP HWDGE ring, B planes the
                # ACT ring, so the two descriptor streams drain in parallel
                nc.sync.dma_start(ar_t[:, :ch, :], a_r[:, ksl, :])
                nc.sync.dma_start(ai_t[:, :ch, :], a_i[:, ksl, :])
                nc.sync.dma_start(an_t[:, :ch, :], a_n[:, ksl, :])
                nc.scalar.dma_start(br_t[:, :ch, :], b_r[:, ksl, :])
                nc.scalar.dma_start(bi_t[:, :ch, :], b_i[:, ksl, :])
                for kk in range(ch):
                    k = k0 + kk
                    start = k == 0
                    stop = k == KT - 1
                    for m in range(MT):
                        msl = slice(m * 128, (m + 1) * 128)
                        lr = ar_t[:, kk, msl]
                        li = ai_t[:, kk, msl]
                        ln = an_t[:, kk, msl]
                        rr = br_t[:, kk, :]
                        ri = bi_t[:, kk, :]
                        # bank_re = sum a_r.b_r + (-a_i).b_i
                        # bank_im = sum a_r.b_i + a_i.b_r
                        nc.tensor.matmul(ps[(m, "re")][:], lr, rr,
                                         start=start, stop=False)
                        nc.tensor.matmul(ps[(m, "im")][:], lr, ri,
                                         start=start, stop=False)
                        nc.tensor.matmul(ps[(m, "re")][:], ln, ri,
                                         start=False, stop=stop)
                        nc.tensor.matmul(ps[(m, "im")][:], li, rr,
                                         start=False, stop=stop)
                k0 += ch

            for m in range(MT):
                msl = slice(m * 128, (m + 1) * 128)
                out_r = o_pool.tile([128, NSH], F32, name="out_r")
                out_i = o_pool.tile([128, NSH], F32, name="out_i")
                nc.vector.tensor_copy(out_r[:], ps[(m, "re")][:])
                nc.vector.tensor_copy(out_i[:], ps[(m, "im")][:])
                nc.sync.dma_start(c_r[msl, :], out_r[:])
                nc.sync.dma_start(c_i[msl, :], out_i[:])

    nc.compile()
    return nc


def _get_nc(dt_name):
    if dt_name not in _CACHE:
        _CACHE[dt_name] = _build(dt_name)
    return _CACHE[dt_name]


def _pack_kxm(mat_t, np_dt):
    # mat_t: [4096, F] (k-major) -> [128, KT, F] with k = kt*128 + p
    f = mat_t.shape[1]
    return np.ascontiguousarray(
        mat_t.reshape(KT, 128, f).transpose(1, 0, 2).astype(np_dt)
    )


def run_device(A, B, dt_name=DT_NAME, trace=False):
    """A: [256, 4096] complex64, B: [4096, 4096] complex64.
    Returns C = A @ B as [256, 4096] complex64 plus the raw results."""
    from concourse import bass_utils

    nc = _get_nc(dt_name)
    np_dt = _np_dtype(dt_name)

    at = A.T  # [4096, 256]
    a_r = _pack_kxm(np.ascontiguousarray(at.real), np_dt)
    a_i = _pack_kxm(np.ascontiguousarray(at.imag), np_dt)
    a_n = np.ascontiguousarray(-a_i)
    br_full = B.real
    bi_full = B.imag

    in_maps = []
    for c in range(N_CORES):
        csl = slice(c * NSH, (c + 1) * NSH)
        in_maps.append({
            "a_r": a_r,
            "a_i": a_i,
            "a_n": a_n,
            "b_r": _pack_kxm(np.ascontiguousarray(br_full[:, csl]), np_dt),
            "b_i": _pack_kxm(np.ascontiguousarray(bi_full[:, csl]), np_dt),
        })

    res = bass_utils.run_bass_kernel_spmd(
        nc, in_maps, core_ids=list(range(N_CORES)), trace=trace
    )

    out = np.empty((BATCH, HALF), dtype=np.complex64)
    for c in range(N_CORES):
        csl = slice(c * NSH, (c + 1) * NSH)
        out.real[:, csl] = res.results[c]["c_r"]
        out.imag[:, csl] = res.results[c]["c_i"]
    return out, res


def kernel(state, target_matrix, control, num_qubits):
    state = np.asarray(state)
    target_matrix = np.asarray(target_matrix)
    control = int(control)
    num_qubits = int(num_qubits)
    dim = 1 << num_qubits

    assert state.shape == (BATCH, DIM) and dim == DIM, (
        "kernel hardcoded for [256, 8192]"
    )

    mask = 1 << (num_qubits - control - 1)
    idx = np.arange(dim)
    c1 = idx[(idx & mask) != 0]  # columns with control bit set

    if control == 0:
        A = state[:, HALF:]
        B = target_matrix[HALF:, HALF:]
    else:
        A = state[:, c1]
        B = target_matrix[np.ix_(c1, c1)]
    A = np.ascontiguousarray(A, dtype=np.complex64)
    B = np.ascontiguousarray(B, dtype=np.complex64)

    C, _ = run_device(A, B)

    out = state.astype(np.complex64, copy=True)
    out[:, c1] = C
    return out
